# revision 4
# baseline (speedup 1.0000x reference)
"""Trainium2 Bass kernel for nn_BaseLoftqLinear (4-bit quantized linear + LoRA + bias).

Computes: out = x @ dequant(W).T + (x @ A.T) @ B.T + bias
  x: [4, 2048, 4096] f32, W: [4096, 4096] 4-bit packed, A: [16, 4096], B: [4096, 16]

Strategy (column-parallel over out_features across 8 cores, pure streaming GEMM):
  - each core owns 512 out_features; x replicated
  - host does ALL weight prep in f32 (unpack 4-bit, codebook lookup, per-block
    absmax scale, LoRA fold (B@A).T), rounds once to bf16, and lays the result
    out chunk-major ([128, KC*N]) so W chunks DMA straight into their SBUF
    residency slots with 4-8KB lines
  - x -> bf16 k-major, re-tiled on host to [128p, g, q, 4*1024] so that one
    partition row of a big x tile is 4 k-chunks x 1024 m CONTIGUOUS (8KB DMA
    lines; the DMA engines are packet-rate-limited at ~190ns/packet/engine,
    so line size sets bandwidth: 2KB lines -> ~170 GB/s, 8KB -> HBM roof)
  - device: out[128m, 512n] += xT[128k,128m].T @ W[128k, 512n] over 32 k-chunks
    per m-tile; bias add on DVE; store f32
  - startup: first 8 k-chunks of x ride the scalar HWDGE ring as small
    [128,1024] tiles (low latency), W chunks the sync ring; the first 6
    m-tiles consume W/x chunks as they land in k-blocks (2,2,4,8,16);
    remaining 58 m-tiles run straight k-inner with big x tiles streaming
    via gpsimd SWDGE
  - host gathers the 8 [8192, 512] outputs -> [4, 2048, 4096]
"""
import os
import sys

for _p in ("/opt/trn_rl_repo", "/root/.axon_site/_ro/trn_rl_repo"):
    if os.path.isdir(_p) and _p not in sys.path:
        sys.path.insert(0, _p)
        break

import numpy as np
import ml_dtypes

import concourse.bass as bass
import concourse.bacc as bacc
import concourse.tile as tile
import concourse.mybir as mybir

dt = mybir.dt

# problem constants (hardcoded per spec)
B_, S_, IN_F, OUT_F, RANK = 4, 2048, 4096, 4096, 16
M = B_ * S_                    # 8192 tokens
N_CORES = 8
N = OUT_F // N_CORES           # 512 out_features per core
BLOCK = 64                     # quant block size (along in_features)
NBLK = IN_F // BLOCK           # 64 scale blocks along k
MT = M // 128                  # 64 m-tiles
KC = IN_F // 128               # 32 k-chunks
GA = 6                         # m-tiles overlapped with W/x startup DMAs
KBLOCKS = [2, 2, 4, 8, 16]     # k-run lengths for group A
SB = 8                         # m-tiles per x superblock
NSB = MT // SB                 # 8 superblocks
XG = 4                         # k-chunks per big x tile (8KB DMA lines)
NQ = KC // XG                  # 8 big x tiles per superblock
XROW = NSB * KC * 1024         # elements per partition row of xt4


def build_program():
    """Single-core Bass program (SPMD: same program on all 8 cores)."""
    nc = bacc.Bacc("TRN2", target_bir_lowering=False, debug=False,
                   num_devices=N_CORES)

    # xt4[p, ((g*NQ + q)*XG + s)*1024 + mm] = x_bf16[g*1024 + mm,
    #                                                (q*XG + s)*128 + p]
    xt4 = nc.dram_tensor("xt4", [128, XROW], dt.bfloat16,
                         kind="ExternalInput")
    # W_eff chunk-major: weff[p, c*N+nn] = W_eff[c*128+p, nn]
    weff = nc.dram_tensor("weff", [128, KC * N], dt.bfloat16,
                          kind="ExternalInput")
    bias = nc.dram_tensor("bias", [N], dt.float32, kind="ExternalInput")
    out = nc.dram_tensor("out", [M, N], dt.float32, kind="ExternalOutput")

    with tile.TileContext(nc) as tc:
        with (
            tc.tile_pool(name="const", bufs=1) as constp,
            tc.tile_pool(name="wt", bufs=1) as wtp,
            tc.tile_pool(name="xsm", bufs=8) as xsp,
            tc.tile_pool(name="xbig", bufs=16) as xbp,
            tc.tile_pool(name="osb", bufs=4) as op_,
            tc.tile_pool(name="ps_out", bufs=8, space="PSUM") as ps_out,
        ):
            xt_t = xt4[:, :].tensor
            small_tiles = {}   # (0, c) -> [128, 1024] tile, c in 0..7
            big_tiles = {}     # (g, q) -> [128, 4096] tile

            def x_small_dma(c, eng):
                """Low-latency startup tile: k-chunk c, m 0..1023 of sb 0."""
                xtile = xsp.tile([128, 1024], dt.bfloat16, tag="xs")
                off = (0 * NQ + c // XG) * (XG * 1024) + (c % XG) * 1024
                src = bass.AP(xt_t, off, [[XROW, 128], [1, 1024]])
                eng.dma_start(out=xtile[:], in_=src)
                small_tiles[(0, c)] = xtile

            def x_big_dma(g, q, eng):
                """Big tile: k-chunks XG*q..XG*q+3 x 1024 m of superblock g."""
                xtile = xbp.tile([128, XG * 1024], dt.bfloat16, tag="xb")
                off = (g * NQ + q) * (XG * 1024)
                src = bass.AP(xt_t, off, [[XROW, 128], [1, XG * 1024]])
                eng.dma_start(out=xtile[:], in_=src)
                big_tiles[(g, q)] = xtile

            def x_slice(g, c, j):
                """lhsT [128k, 128m] for k-chunk c, local m-tile j."""
                if (g, c) in small_tiles:
                    ap = small_tiles[(g, c)][:]
                    off = j * 128
                else:
                    ap = big_tiles[(g, c // XG)][:]
                    off = (c % XG) * 1024 + j * 128
                return bass.AP(ap.tensor, ap.offset + off,
                               [list(ap.ap[0]), [1, 128]])

            # W_eff resident: wt_sb[:, c*N + nn] = W_eff[c*128 + p, nn]
            wt_sb = wtp.tile([128, KC * N], dt.bfloat16, name="wt_sb")
            bias_sb = constp.tile([128, N], dt.float32, name="bias_sb")

            # ---- startup DMAs: small x on scalar ring, W on sync ring ----
            x_small_dma(0, nc.scalar)
            nc.sync.dma_start(out=wt_sb[:, 0:2 * N], in_=weff[:, 0:2 * N])
            x_small_dma(1, nc.scalar)
            nc.sync.dma_start(out=wt_sb[:, 2 * N:4 * N],
                              in_=weff[:, 2 * N:4 * N])
            x_small_dma(2, nc.scalar)
            x_small_dma(3, nc.scalar)
            nc.sync.dma_start(out=wt_sb[:, 4 * N:8 * N],
                              in_=weff[:, 4 * N:8 * N])
            bsrc = bass.AP(bias[:].tensor, 0, [[0, 128], [1, N]])
            nc.scalar.dma_start(out=bias_sb[:], in_=bsrc)
            for c in (4, 5, 6, 7):
                x_small_dma(c, nc.scalar)
            for c0 in range(8, KC, 8):
                nc.sync.dma_start(out=wt_sb[:, c0 * N:(c0 + 8) * N],
                                  in_=weff[:, c0 * N:(c0 + 8) * N])
            for q in range(2, NQ):
                x_big_dma(0, q, nc.gpsimd)
            for q in range(NQ):
                x_big_dma(1, q, nc.gpsimd)

            po_A = []
            for _j in range(GA):
                poa = ps_out.tile([128, N], dt.float32, tag="po")
                po_A.append(poa)

            def emit_block(c0, c1):
                """k-run [c0, c1) for each of the GA early m-tiles."""
                for j in range(GA):
                    for c in range(c0, c1):
                        nc.tensor.matmul(
                            po_A[j][:],
                            x_slice(0, c, j),
                            wt_sb[:, c * N:(c + 1) * N],
                            start=(c == 0), stop=(c == KC - 1),
                        )

            e = 0
            for b in KBLOCKS:
                emit_block(e, e + b)
                e += b

            def store(ms, po, split=1):
                o_sb = op_.tile([128, N], dt.float32, tag="o_sb")
                pp = 128 // split
                for h in range(split):
                    nc.vector.tensor_tensor(
                        o_sb[h * pp:(h + 1) * pp, :],
                        po[h * pp:(h + 1) * pp, :],
                        bias_sb[h * pp:(h + 1) * pp, :],
                        mybir.AluOpType.add)
                    nc.sync.dma_start(
                        out=out[ms * 128 + h * pp:ms * 128 + (h + 1) * pp, :],
                        in_=o_sb[h * pp:(h + 1) * pp, :])

            for j in range(GA):
                store(j, po_A[j])

            # ---- tail: k-inner m-tiles with resident W_eff ----
            for ms in range(GA, MT):
                g, j = ms // SB, ms % SB
                # prefetch one big tile of superblock g+1 per m-tile
                if g + 1 < NSB and (g + 1, j) not in big_tiles:
                    x_big_dma(g + 1, j, nc.gpsimd)
                po = ps_out.tile([128, N], dt.float32, tag="po")
                for c in range(KC):
                    nc.tensor.matmul(
                        po[:],
                        x_slice(g, c, j),
                        wt_sb[:, c * N:(c + 1) * N],
                        start=(c == 0), stop=(c == KC - 1),
                    )
                store(ms, po, split=2 if ms == MT - 1 else 1)

    nc.compile()
    return nc


_cache = {}


def _get_program(lookup_table=None):
    # program is independent of input values
    if "nc" not in _cache:
        _cache["nc"] = build_program()
    return _cache["nc"]


def make_in_maps(inputs: dict):
    lut = np.asarray(inputs["lookup_table"], dtype=np.float32)

    x = np.asarray(inputs["x"], dtype=np.float32).reshape(M, IN_F)
    xb = x.astype(ml_dtypes.bfloat16)
    # xt4[p, g, q, s, mm] = x[g*1024+mm, (q*XG+s)*128+p]
    xt4 = np.ascontiguousarray(
        xb.reshape(NSB, 1024, NQ, XG, 128).transpose(4, 0, 2, 3, 1)
    ).reshape(128, XROW)

    pk_full = np.asarray(inputs["packed_qweight"]).astype(np.uint8).reshape(-1)
    idx_full = np.empty(pk_full.size * 2, np.uint8)
    idx_full[0::2] = pk_full & 15
    idx_full[1::2] = pk_full >> 4
    idx_full = idx_full.reshape(OUT_F, IN_F)

    wmax_full = np.asarray(inputs["weight_max"], dtype=np.float32).reshape(OUT_F, NBLK)
    lora_a = np.asarray(inputs["lora_A"], dtype=np.float32)
    lora_b = np.asarray(inputs["lora_B"], dtype=np.float32)
    bias_full = np.asarray(inputs["bias"], dtype=np.float32).reshape(-1)

    # full weight prep in f32, one rounding to bf16 at the end
    wf = lut[idx_full]                                    # [OUT_F, IN_F]
    wf = wf.reshape(OUT_F, NBLK, BLOCK) * wmax_full[:, :, None]
    wf = wf.reshape(OUT_F, IN_F) + lora_b @ lora_a        # LoRA fold

    in_maps = []
    for i in range(N_CORES):
        o0, o1 = i * N, (i + 1) * N
        wt = wf[o0:o1, :].T                               # [IN_F, N]
        weff = np.ascontiguousarray(
            wt.reshape(KC, 128, N).transpose(1, 0, 2).reshape(128, KC * N)
        ).astype(ml_dtypes.bfloat16)
        in_maps.append({
            "xt4": xt4,
            "weff": weff,
            "bias": bias_full[o0:o1],
        })
    return in_maps


def kernel(**inputs) -> np.ndarray:
    from concourse.bass_utils import run_bass_kernel_spmd

    nc = _get_program()
    in_maps = make_in_maps(inputs)
    res = run_bass_kernel_spmd(nc, in_maps, core_ids=list(range(N_CORES)))
    outs = [np.asarray(r["out"], dtype=np.float32) for r in res.results]
    full = np.concatenate(outs, axis=1)  # [M, OUT_F]
    return full.reshape(B_, S_, OUT_F)


# revision 7
# speedup vs baseline: 1.0263x; 1.0263x over previous
"""Trainium2 Bass kernel for nn_BaseLoftqLinear (4-bit quantized linear + LoRA + bias).

Computes: out = x @ dequant(W).T + (x @ A.T) @ B.T + bias
  x: [4, 2048, 4096] f32, W: [4096, 4096] 4-bit packed, A: [16, 4096], B: [4096, 16]

Strategy (column-parallel over out_features across 8 cores, pure streaming GEMM):
  - each core owns 512 out_features; x replicated
  - host does ALL weight prep in f32 (unpack 4-bit, codebook lookup, per-block
    absmax scale, LoRA fold (B@A).T), rounds once to bf16, chunk-major layout
  - x -> bf16 k-major, re-tiled on host to [128p, g, q, 4*1024] so one
    partition row of an x tile is 4 k-chunks x 1024 m CONTIGUOUS = 8KB DMA
    lines.  DMA queues dispatch ~1 packet/slot round-robin, so bandwidth
    share is proportional to line size: 8KB lines everywhere, and x striped
    over THREE queues (scalar/gpsimd/vector) during the startup chase
  - device: out[128m, 512n] += xT[128k,128m].T @ W[128k, 512n] over 32 k-chunks
    per m-tile; bias add on DVE; store f32
  - startup: first two x tiles and first W group are split half-partition
    across two queues each to halve first-arrival latency; the first 6
    m-tiles consume chunks as they land in k-blocks (4,4,8,16); remaining
    58 m-tiles run straight k-inner
  - host gathers the 8 [8192, 512] outputs -> [4, 2048, 4096]
"""
import os
import sys

for _p in ("/opt/trn_rl_repo", "/root/.axon_site/_ro/trn_rl_repo"):
    if os.path.isdir(_p) and _p not in sys.path:
        sys.path.insert(0, _p)
        break

import numpy as np
import ml_dtypes

import concourse.bass as bass
import concourse.bacc as bacc
import concourse.tile as tile
import concourse.mybir as mybir

dt = mybir.dt

# problem constants (hardcoded per spec)
B_, S_, IN_F, OUT_F, RANK = 4, 2048, 4096, 4096, 16
M = B_ * S_                    # 8192 tokens
N_CORES = 8
N = OUT_F // N_CORES           # 512 out_features per core
BLOCK = 64                     # quant block size (along in_features)
NBLK = IN_F // BLOCK           # 64 scale blocks along k
MT = M // 128                  # 64 m-tiles
KC = IN_F // 128               # 32 k-chunks
GA = 6                         # m-tiles overlapped with W/x startup DMAs
KBLOCKS = [4, 4, 8, 16]        # k-run lengths for group A
SB = 8                         # m-tiles per x superblock
NSB = MT // SB                 # 8 superblocks
XG = 4                         # k-chunks per x tile (8KB DMA lines)
NQ = KC // XG                  # 8 x tiles per superblock
XROW = NSB * KC * 1024         # elements per partition row of xt4


def build_program():
    """Single-core Bass program (SPMD: same program on all 8 cores)."""
    nc = bacc.Bacc("TRN2", target_bir_lowering=False, debug=False,
                   num_devices=N_CORES)

    # xt4[p, ((g*NQ + q)*XG + s)*1024 + mm] = x_bf16[g*1024 + mm,
    #                                                (q*XG + s)*128 + p]
    xt4 = nc.dram_tensor("xt4", [128, XROW], dt.bfloat16,
                         kind="ExternalInput")
    # W_eff chunk-major: weff[p, c*N+nn] = W_eff[c*128+p, nn]
    weff = nc.dram_tensor("weff", [128, KC * N], dt.bfloat16,
                          kind="ExternalInput")
    bias = nc.dram_tensor("bias", [N], dt.float32, kind="ExternalInput")
    out = nc.dram_tensor("out", [M, N], dt.float32, kind="ExternalOutput")

    with tile.TileContext(nc) as tc:
        with (
            tc.tile_pool(name="const", bufs=1) as constp,
            tc.tile_pool(name="wt", bufs=1) as wtp,
            tc.tile_pool(name="xbig", bufs=16) as xbp,
            tc.tile_pool(name="osb", bufs=4) as op_,
            tc.tile_pool(name="ps_out", bufs=8, space="PSUM") as ps_out,
        ):
            xt_t = xt4[:, :].tensor
            big_tiles = {}     # (g, q) -> [128, 4096] tile

            def x_big_dma(g, q, eng, eng2=None):
                """x tile: k-chunks XG*q..XG*q+3 x 1024 m of superblock g.
                With eng2, split half-partition across two queues."""
                xtile = xbp.tile([128, XG * 1024], dt.bfloat16, tag="xb")
                off = (g * NQ + q) * (XG * 1024)
                if eng2 is None:
                    src = bass.AP(xt_t, off, [[XROW, 128], [1, XG * 1024]])
                    eng.dma_start(out=xtile[:], in_=src)
                else:
                    s0 = bass.AP(xt_t, off, [[XROW, 64], [1, XG * 1024]])
                    s1 = bass.AP(xt_t, off + 64 * XROW,
                                 [[XROW, 64], [1, XG * 1024]])
                    eng.dma_start(out=xtile[0:64, :], in_=s0)
                    eng2.dma_start(out=xtile[64:128, :], in_=s1)
                big_tiles[(g, q)] = xtile

            def x_slice(g, c, j):
                """lhsT [128k, 128m] for k-chunk c, local m-tile j."""
                ap = big_tiles[(g, c // XG)][:]
                off = (c % XG) * 1024 + j * 128
                return bass.AP(ap.tensor, ap.offset + off,
                               [list(ap.ap[0]), [1, 128]])

            # W_eff resident: wt_sb[:, c*N + nn] = W_eff[c*128 + p, nn]
            wt_sb = wtp.tile([128, KC * N], dt.bfloat16, name="wt_sb")
            bias_sb = constp.tile([128, N], dt.float32, name="bias_sb")

            W8 = 8 * N  # one 8-chunk W group (8KB lines)

            # ---- startup DMAs ----
            x_big_dma(0, 0, nc.scalar, nc.gpsimd)
            # first W group small (4 chunks) so it doesn't gate block 1
            nc.sync.dma_start(out=wt_sb[:, 0:4 * N], in_=weff[:, 0:4 * N])
            x_big_dma(0, 1, nc.scalar, nc.gpsimd)
            nc.sync.dma_start(out=wt_sb[:, 4 * N:8 * N],
                              in_=weff[:, 4 * N:8 * N])
            bsrc = bass.AP(bias[:].tensor, 0, [[0, 128], [1, N]])
            nc.scalar.dma_start(out=bias_sb[:], in_=bsrc)
            rr = [nc.scalar, nc.gpsimd]
            for i, q in enumerate(range(2, NQ)):
                x_big_dma(0, q, rr[i % 2])
            for g8 in range(1, 4):
                nc.sync.dma_start(out=wt_sb[:, g8 * W8:(g8 + 1) * W8],
                                  in_=weff[:, g8 * W8:(g8 + 1) * W8])
            for i, q in enumerate(range(NQ)):
                x_big_dma(1, q, rr[i % 2])

            po_A = []
            for _j in range(GA):
                poa = ps_out.tile([128, N], dt.float32, tag="po")
                po_A.append(poa)

            def emit_block(c0, c1):
                """k-run [c0, c1) for each of the GA early m-tiles."""
                for j in range(GA):
                    for c in range(c0, c1):
                        nc.tensor.matmul(
                            po_A[j][:],
                            x_slice(0, c, j),
                            wt_sb[:, c * N:(c + 1) * N],
                            start=(c == 0), stop=(c == KC - 1),
                        )

            e = 0
            for b in KBLOCKS:
                emit_block(e, e + b)
                e += b

            def store(ms, po, split=1):
                o_sb = op_.tile([128, N], dt.float32, tag="o_sb")
                pp = 128 // split
                for h in range(split):
                    nc.vector.tensor_tensor(
                        o_sb[h * pp:(h + 1) * pp, :],
                        po[h * pp:(h + 1) * pp, :],
                        bias_sb[h * pp:(h + 1) * pp, :],
                        mybir.AluOpType.add)
                    nc.sync.dma_start(
                        out=out[ms * 128 + h * pp:ms * 128 + (h + 1) * pp, :],
                        in_=o_sb[h * pp:(h + 1) * pp, :])

            for j in range(GA):
                store(j, po_A[j])

            # ---- tail: k-inner m-tiles with resident W_eff ----
            for ms in range(GA, MT):
                g, j = ms // SB, ms % SB
                # prefetch one x tile of superblock g+1 per m-tile
                if g + 1 < NSB and (g + 1, j) not in big_tiles:
                    x_big_dma(g + 1, j, rr[j % 2])
                po = ps_out.tile([128, N], dt.float32, tag="po")
                for c in range(KC):
                    nc.tensor.matmul(
                        po[:],
                        x_slice(g, c, j),
                        wt_sb[:, c * N:(c + 1) * N],
                        start=(c == 0), stop=(c == KC - 1),
                    )
                store(ms, po, split=2 if ms == MT - 1 else 1)

    nc.compile()
    return nc


_cache = {}


def _get_program(lookup_table=None):
    # program is independent of input values
    if "nc" not in _cache:
        _cache["nc"] = build_program()
    return _cache["nc"]


def make_in_maps(inputs: dict):
    x = np.asarray(inputs["x"], dtype=np.float32).reshape(M, IN_F)
    xb = x.astype(ml_dtypes.bfloat16)
    # xt4[p, g, q, s, mm] = x[g*1024+mm, (q*XG+s)*128+p]
    xt4 = np.ascontiguousarray(
        xb.reshape(NSB, 1024, NQ, XG, 128).transpose(4, 0, 2, 3, 1)
    ).reshape(128, XROW)

    lut = np.asarray(inputs["lookup_table"], dtype=np.float32)
    pk_full = np.asarray(inputs["packed_qweight"]).astype(np.uint8).reshape(-1)
    idx_full = np.empty(pk_full.size * 2, np.uint8)
    idx_full[0::2] = pk_full & 15
    idx_full[1::2] = pk_full >> 4
    idx_full = idx_full.reshape(OUT_F, IN_F)

    wmax_full = np.asarray(inputs["weight_max"], dtype=np.float32).reshape(OUT_F, NBLK)
    lora_a = np.asarray(inputs["lora_A"], dtype=np.float32)
    lora_b = np.asarray(inputs["lora_B"], dtype=np.float32)
    bias_full = np.asarray(inputs["bias"], dtype=np.float32).reshape(-1)

    # full weight prep in f32, one rounding to bf16 at the end
    wf = lut[idx_full]                                    # [OUT_F, IN_F]
    wf = wf.reshape(OUT_F, NBLK, BLOCK) * wmax_full[:, :, None]
    wf = wf.reshape(OUT_F, IN_F) + lora_b @ lora_a        # LoRA fold

    in_maps = []
    for i in range(N_CORES):
        o0, o1 = i * N, (i + 1) * N
        wt = wf[o0:o1, :].T                               # [IN_F, N]
        weff = np.ascontiguousarray(
            wt.reshape(KC, 128, N).transpose(1, 0, 2).reshape(128, KC * N)
        ).astype(ml_dtypes.bfloat16)
        in_maps.append({
            "xt4": xt4,
            "weff": weff,
            "bias": bias_full[o0:o1],
        })
    return in_maps


def kernel(**inputs) -> np.ndarray:
    from concourse.bass_utils import run_bass_kernel_spmd

    nc = _get_program()
    in_maps = make_in_maps(inputs)
    res = run_bass_kernel_spmd(nc, in_maps, core_ids=list(range(N_CORES)))
    outs = [np.asarray(r["out"], dtype=np.float32) for r in res.results]
    full = np.concatenate(outs, axis=1)  # [M, OUT_F]
    return full.reshape(B_, S_, OUT_F)
